# revision 32
# baseline (speedup 1.0000x reference)
# Trainium2 Bass kernel for nn_AttentionalPropagation (B=2, D=256, N=M=4096, H=4).
#
# Sharding: 8 cores; each batch (B=2) owns 4 cores; each core computes a
# 1024-column sequence shard of the output end-to-end (q/scores/softmax/attn/
# message/MLP). k,v are computed redundantly per core from the full `source`
# of its batch. The only cross-core communication is an AllReduce of the
# InstanceNorm partial (sum, sumsq) statistics within each 4-core batch group.
#
# Per-head layout trick: conv weights' output channels are permuted host-side
# so that head channels are contiguous on SBUF partitions (head h lives at
# partitions 64*(h%2) of channel-chunk h//2); this lets per-head matmuls run
# directly off partition-aligned slices (PE row/col tiling).
#
# Softmax: scores are built transposed ([m, n] with m on partitions), exp'd on
# ScalarE (scale=1/8 folded in, no max-subtraction needed: |s/8| < ~5), and the
# softmax denominator is obtained for free by augmenting v^T with a ones
# column in the attn matmul (out row 64 = sum_m exp). Normalization is a
# per-column reciprocal broadcast multiply.

import numpy as np

import concourse.bass as bass  # noqa: F401  (bass types used via tile/bacc)
import concourse.tile as tile
import concourse.mybir as mybir
from concourse import bacc
from concourse import bass_utils

B, D, N = 2, 256, 4096
H, DH = 4, 64
NS = N // 4           # sequence shard per core
NCORES = 8
EPS = 1e-5

FP = mybir.dt.float32
BF = mybir.dt.bfloat16
F8 = mybir.dt.float8e4
AX = mybir.AxisListType
OP = mybir.AluOpType
AF = mybir.ActivationFunctionType

# j-groups for the scores->exp pipeline. Each group's PSUM tile holds BOTH
# heads of the current pair (2 x glen x 512 fp32): glen=2 -> 4 banks,
# glen=1 -> 2 banks; the two tiles double-buffer within 6 free banks and the
# two per-head attn accumulators take the other 2.
_JGROUPS = []
_j = 0
while _j < 32:
    g = 2 if (len(_JGROUPS) % 2 == 0 and _j + 2 <= 32) else 1
    _JGROUPS.append((_j, g))
    _j += g

import os
_STAGE = os.environ.get("KSTAGE", "full")  # debug bisection: proj|attn|nocc|full


def _emit(nc, tc, io, es):
    xs, src = io["xs"], io["src"]
    out = io["out"]

    wpool = es.enter_context(tc.tile_pool(name="weights", bufs=1))
    apool = es.enter_context(tc.tile_pool(name="acts", bufs=1))

    # ---------- weight / bias loads ----------
    wq_sb = wpool.tile([128, 2, D], FP)
    nc.sync.dma_start(out=wq_sb[:], in_=io["wqT"].rearrange("(c p) o -> p c o", p=128))
    wk_sb = wpool.tile([128, 2, D], FP)
    nc.sync.dma_start(out=wk_sb[:], in_=io["wkT"].rearrange("(c p) o -> p c o", p=128))
    wv_sb = wpool.tile([128, 2, D], FP)
    nc.sync.dma_start(out=wv_sb[:], in_=io["wvT"].rearrange("(c p) o -> p c o", p=128))
    wm_sb = wpool.tile([128, 2, D], BF)
    nc.gpsimd.dma_start(out=wm_sb[:], in_=io["wmT"].rearrange("(c p) o -> p c o", p=128))
    w1x_sb = wpool.tile([128, 2, 2 * D], FP)
    nc.sync.dma_start(out=w1x_sb[:], in_=io["w1xT"].rearrange("(c p) o -> p c o", p=128))
    w1m_sb = wpool.tile([128, 2, 2 * D], BF)
    nc.gpsimd.dma_start(out=w1m_sb[:], in_=io["w1mT"].rearrange("(c p) o -> p c o", p=128))
    w2_sb = wpool.tile([128, 4, D], BF)
    nc.gpsimd.dma_start(out=w2_sb[:], in_=io["w2T"].rearrange("(c p) o -> p c o", p=128))

    bq_sb = wpool.tile([128, 2], FP)
    nc.sync.dma_start(out=bq_sb[:], in_=io["bq"][:])
    bk_sb = wpool.tile([128, 2], FP)
    nc.sync.dma_start(out=bk_sb[:], in_=io["bk"][:])
    bm_sb = wpool.tile([128, 2], FP)
    nc.sync.dma_start(out=bm_sb[:], in_=io["bm"][:])
    b1_sb = wpool.tile([128, 4], FP)
    nc.sync.dma_start(out=b1_sb[:], in_=io["b1"][:])
    b2_sb = wpool.tile([128, 2], FP)
    nc.sync.dma_start(out=b2_sb[:], in_=io["b2"][:])
    bv_sb = wpool.tile([1, D], FP)
    nc.sync.dma_start(out=bv_sb[:], in_=io["bv"][:])
    bvb_sb = wpool.tile([128, D], FP)
    nc.gpsimd.partition_broadcast(bvb_sb[:], bv_sb[:])

    xs_sb = apool.tile([128, 2, NS], FP)
    nc.sync.dma_start(out=xs_sb[:], in_=xs.rearrange("(c p) n -> p c n", p=128))

    # ---------- persistent activation tiles ----------
    q_sb = apool.tile([128, 2, NS], BF)
    k_sb = apool.tile([128, 2, N], BF)
    # v^T per head + ones col, fp8, padded to stride 80 for DoubleRow
    vaT_sb = apool.tile([128, H, 16, 2, 80], F8)
    exp_sb = apool.tile([128, 2, 32, 512], F8)   # [., head-of-pair, m-chunk, n]
    attn_sb = apool.tile([128, 2, NS], BF)
    msg_sb = apool.tile([128, 2, NS], BF)
    h1_sb = apool.tile([128, 4, NS], FP)
    h1n_sb = apool.tile([128, 4, NS], BF)
    out_sb = apool.tile([128, 2, NS], FP)
    stats_sb = apool.tile([128, 8], FP)

    nc.vector.memset(vaT_sb[:, :, :, :, DH:DH + 1], 1.0)

    # ---------- phase 1: projections ----------
    with tc.tile_pool(name="srcp", bufs=1) as srcpool, \
         tc.tile_pool(name="pj", bufs=4, space="PSUM") as pj, \
         tc.tile_pool(name="vt", bufs=3, space="PSUM") as vtp:
        src_sb = srcpool.tile([128, 2, N], FP)
        nc.sync.dma_start(out=src_sb[:], in_=src.rearrange("(c p) m -> p c m", p=128))

        # q = WqT.T @ xs + bq   [256, NS]
        for oc in range(2):
            for ns in range(NS // 512):
                q_ps = pj.tile([128, 512], FP, tag="pj")
                for ic in range(2):
                    nc.tensor.matmul(
                        q_ps[:],
                        wq_sb[:, ic, oc * 128:(oc + 1) * 128],
                        xs_sb[:, ic, ns * 512:(ns + 1) * 512],
                        start=(ic == 0), stop=(ic == 1),
                    )
                nc.vector.tensor_scalar_add(
                    q_sb[:, oc, ns * 512:(ns + 1) * 512], q_ps[:], bq_sb[:, oc:oc + 1])

        # k = WkT.T @ src + bk   [256, N]
        for oc in range(2):
            for ns in range(N // 512):
                k_ps = pj.tile([128, 512], FP, tag="pj")
                for ic in range(2):
                    nc.tensor.matmul(
                        k_ps[:],
                        wk_sb[:, ic, oc * 128:(oc + 1) * 128],
                        src_sb[:, ic, ns * 512:(ns + 1) * 512],
                        start=(ic == 0), stop=(ic == 1),
                    )
                nc.vector.tensor_scalar_add(
                    k_sb[:, oc, ns * 512:(ns + 1) * 512], k_ps[:], bk_sb[:, oc:oc + 1])

        # v^T (+bias) directly transposed: out[m, c] = sum_i src[i, m] WvT[i, c]
        for mc in range(N // 128):
            vt_ps = vtp.tile([128, D], FP, tag="vt")
            for ic in range(2):
                nc.tensor.matmul(
                    vt_ps[:],
                    src_sb[:, ic, mc * 128:(mc + 1) * 128],
                    wv_sb[:, ic, :],
                    start=(ic == 0), stop=(ic == 1),
                )
            for h in range(H):
                nc.vector.tensor_add(
                    vaT_sb[:, h, mc // 2, mc % 2, 0:DH],
                    vt_ps[:, h * DH:(h + 1) * DH],
                    bvb_sb[:, h * DH:(h + 1) * DH],
                )

    if _STAGE == "proj":
        nc.vector.tensor_copy(out_sb[:], q_sb[:])
        nc.sync.dma_start(out=out.rearrange("(c p) n -> p c n", p=128), in_=out_sb[:])
        return

    # ---------- phase 2: attention ----------
    with tc.tile_pool(name="scA", bufs=1, space="PSUM") as scA, \
         tc.tile_pool(name="scB", bufs=1, space="PSUM") as scB, \
         tc.tile_pool(name="at", bufs=1, space="PSUM") as atp, \
         tc.tile_pool(name="nrm", bufs=4) as nrm:
        for hp in range(2):
            kc = hp
            for nch in range(NS // 512):
                n0 = nch * 512
                # scores_T[m, n] = k_h[:, m].T @ q_h[:, n] ; exp on ScalarE.
                # The pair's heads sit at base partitions 0/64, so adjacent
                # matmuls target disjoint PE row groups and run concurrently.
                for (j0, glen) in _JGROUPS:
                    pool = scA if glen == 2 else scB
                    sc_ps = pool.tile([128, 2, glen, 512], FP, tag=pool.name)
                    for j4 in range(glen):
                        j = j0 + j4
                        for hh in range(2):
                            bp = 64 * hh
                            nc.tensor.matmul(
                                sc_ps[:, hh, j4, :],
                                k_sb[bp:bp + DH, kc, j * 128:(j + 1) * 128],
                                q_sb[bp:bp + DH, kc, n0:n0 + 512],
                                start=True, stop=True,
                            )
                    nc.scalar.activation(
                        out=exp_sb[:, :, j0:j0 + glen, :], in_=sc_ps[:],
                        func=AF.Exp, scale=0.125)
                # attn (+Z) accumulate: out[0:64]=sum_m vT*exp, out[64]=sum_m exp
                # fp8 DoubleRow: two 128-row m-chunks per matmul pass
                for hh in range(2):
                    h, bp = 2 * hp + hh, 64 * hh
                    at_ps = atp.tile([128, 512], FP, tag=f"at{hh}")
                    for p in range(16):
                        nc.tensor.matmul(
                            at_ps[:DH + 1, :],
                            vaT_sb[:, h, p, :, 0:DH + 1],
                            exp_sb[:, hh, 2 * p:2 * p + 2, :],
                            start=(p == 0), stop=(p == 15),
                            perf_mode=mybir.MatmulPerfMode.DoubleRow,
                        )
                    rz = nrm.tile([1, 512], FP, tag="rz")
                    nc.vector.reciprocal(rz[:], at_ps[DH:DH + 1, :])
                    rzb = nrm.tile([DH, 512], FP, tag="rzb")
                    nc.gpsimd.partition_broadcast(rzb[:], rz[:])
                    nc.vector.tensor_mul(
                        attn_sb[bp:bp + DH, kc, n0:n0 + 512], at_ps[0:DH, :], rzb[:])

    if _STAGE == "attn":
        nc.vector.tensor_copy(out_sb[:], attn_sb[:])
        nc.sync.dma_start(out=out.rearrange("(c p) n -> p c n", p=128), in_=out_sb[:])
        return

    # ---------- phase 3: message, MLP, instance norm, output ----------
    with tc.tile_pool(name="mm", bufs=6, space="PSUM") as mm, \
         tc.tile_pool(name="dram", bufs=1, space="DRAM") as dram, \
         tc.tile_pool(name="nstat", bufs=1) as nstat:
        # message = WmT.T @ attn + bm
        for oc in range(2):
            for ns in range(NS // 512):
                m_ps = mm.tile([128, 512], FP, tag="mm")
                for ic in range(2):
                    nc.tensor.matmul(
                        m_ps[:],
                        wm_sb[:, ic, oc * 128:(oc + 1) * 128],
                        attn_sb[:, ic, ns * 512:(ns + 1) * 512],
                        start=(ic == 0), stop=(ic == 1),
                    )
                nc.vector.tensor_scalar_add(
                    msg_sb[:, oc, ns * 512:(ns + 1) * 512], m_ps[:], bm_sb[:, oc:oc + 1])

        if _STAGE == "msg":
            nc.vector.tensor_copy(out_sb[:], msg_sb[:])
            nc.sync.dma_start(out=out.rearrange("(c p) n -> p c n", p=128), in_=out_sb[:])
            return

        # h1 = W1T.T @ [xs; msg] + b1   [512, NS]
        for oc in range(4):
            for ns in range(NS // 512):
                h_ps = mm.tile([128, 512], FP, tag="mm")
                for ic in range(2):
                    nc.tensor.matmul(
                        h_ps[:],
                        w1x_sb[:, ic, oc * 128:(oc + 1) * 128],
                        xs_sb[:, ic, ns * 512:(ns + 1) * 512],
                        start=(ic == 0), stop=False,
                    )
                for ic in range(2):
                    nc.tensor.matmul(
                        h_ps[:],
                        w1m_sb[:, ic, oc * 128:(oc + 1) * 128],
                        msg_sb[:, ic, ns * 512:(ns + 1) * 512],
                        start=False, stop=(ic == 1),
                    )
                nc.vector.tensor_scalar_add(
                    h1_sb[:, oc, ns * 512:(ns + 1) * 512], h_ps[:], b1_sb[:, oc:oc + 1])

        if _STAGE == "h1":
            nc.vector.tensor_copy(out_sb[:, 0, :], h1_sb[:, 0, :])
            nc.vector.tensor_copy(out_sb[:, 1, :], h1_sb[:, 1, :])
            nc.sync.dma_start(out=out.rearrange("(c p) n -> p c n", p=128), in_=out_sb[:])
            return

        # per-core partial stats (sum, sumsq) over the local NS columns,
        # via bn_stats/bn_aggr (mean, biased var) -> scaled to (sum, sumsq)
        for t in range(4):
            bst = nstat.tile([128, 2, 6], FP, tag="bst")
            for g in range(2):
                nc.vector.bn_stats(out=bst[:, g, :], in_=h1_sb[:, t, g * 512:(g + 1) * 512])
            mv = nstat.tile([128, 2], FP, tag="mv")
            nc.vector.bn_aggr(out=mv[:], in_=bst[:])
            nc.vector.tensor_scalar_mul(stats_sb[:, t:t + 1], mv[:, 0:1], float(NS))
            msq = nstat.tile([128, 1], FP, tag="msq")
            nc.vector.tensor_mul(msq[:], mv[:, 0:1], mv[:, 0:1])
            msq2 = nstat.tile([128, 1], FP, tag="msq2")
            nc.vector.tensor_add(msq2[:], mv[:, 1:2], msq[:])
            nc.vector.tensor_scalar_mul(stats_sb[:, 4 + t:5 + t], msq2[:], float(NS))

        if _STAGE == "stats":
            nc.vector.tensor_copy(out_sb[:, 0, :], h1_sb[:, 0, :])
            nc.vector.tensor_copy(out_sb[:, 1, 0:8], stats_sb[:])
            nc.sync.dma_start(out=out.rearrange("(c p) n -> p c n", p=128), in_=out_sb[:])
            return

        # cross-core reduce within each batch group of 4 cores
        sred = nstat.tile([128, 8], FP)
        if _STAGE == "nocc":
            nc.vector.tensor_scalar_mul(sred[:], stats_sb[:], 4.0)
        else:
            cc_in = dram.tile([128, 8], FP)
            cc_out = dram.tile([128, 8], FP)
            nc.sync.dma_start(out=cc_in[:], in_=stats_sb[:])
            nc.gpsimd.collective_compute(
                "AllReduce", OP.add,
                replica_groups=[[0, 1, 2, 3], [4, 5, 6, 7]],
                ins=[cc_in[:].opt()], outs=[cc_out[:].opt()],
            )
            nc.sync.dma_start(out=sred[:], in_=cc_out[:])

        mu4 = nstat.tile([128, 4], FP)
        nc.vector.tensor_scalar_mul(mu4[:], sred[:, 0:4], 1.0 / N)
        e24 = nstat.tile([128, 4], FP)
        nc.vector.tensor_scalar_mul(e24[:], sred[:, 4:8], 1.0 / N)
        var4 = nstat.tile([128, 4], FP)
        nc.vector.tensor_mul(var4[:], mu4[:], mu4[:])
        nc.vector.tensor_tensor(out=var4[:], in0=e24[:], in1=var4[:], op=OP.subtract)
        eps1 = nstat.tile([128, 1], FP)
        nc.vector.memset(eps1[:], EPS)
        std4 = nstat.tile([128, 4], FP)
        nc.scalar.activation(out=std4[:], in_=var4[:], func=AF.Sqrt, bias=eps1[:])
        rstd4 = nstat.tile([128, 4], FP)
        nc.vector.reciprocal(rstd4[:], std4[:])
        nb4 = nstat.tile([128, 4], FP)
        nc.vector.tensor_mul(nb4[:], mu4[:], rstd4[:])
        nc.vector.tensor_scalar_mul(nb4[:], nb4[:], -1.0)

        # h = relu((h1 - mu) * rstd) = relu(h1 * rstd - mu * rstd)
        for t in range(4):
            nc.scalar.activation(
                out=h1n_sb[:, t, :], in_=h1_sb[:, t, :], func=AF.Relu,
                bias=nb4[:, t:t + 1], scale=rstd4[:, t:t + 1])

        # out = W2T.T @ h + b2
        for oc in range(2):
            for ns in range(NS // 512):
                o_ps = mm.tile([128, 512], FP, tag="mm")
                for kc2 in range(4):
                    nc.tensor.matmul(
                        o_ps[:],
                        w2_sb[:, kc2, oc * 128:(oc + 1) * 128],
                        h1n_sb[:, kc2, ns * 512:(ns + 1) * 512],
                        start=(kc2 == 0), stop=(kc2 == 3),
                    )
                nc.vector.tensor_scalar_add(
                    out_sb[:, oc, ns * 512:(ns + 1) * 512], o_ps[:], b2_sb[:, oc:oc + 1])

        nc.sync.dma_start(out=out.rearrange("(c p) n -> p c n", p=128), in_=out_sb[:])


_BUILT = {}


def _build():
    if "nc" in _BUILT:
        return _BUILT["nc"]
    nc = bacc.Bacc("TRN2", target_bir_lowering=False, debug=False,
                   enable_asserts=True, num_devices=NCORES)
    io = {}
    io["xs"] = nc.dram_tensor("xs", [D, NS], FP, kind="ExternalInput").ap()
    io["src"] = nc.dram_tensor("src", [D, N], FP, kind="ExternalInput").ap()
    io["wqT"] = nc.dram_tensor("wqT", [D, D], FP, kind="ExternalInput").ap()
    io["wkT"] = nc.dram_tensor("wkT", [D, D], FP, kind="ExternalInput").ap()
    io["wvT"] = nc.dram_tensor("wvT", [D, D], FP, kind="ExternalInput").ap()
    io["wmT"] = nc.dram_tensor("wmT", [D, D], FP, kind="ExternalInput").ap()
    io["w1xT"] = nc.dram_tensor("w1xT", [D, 2 * D], FP, kind="ExternalInput").ap()
    io["w1mT"] = nc.dram_tensor("w1mT", [D, 2 * D], FP, kind="ExternalInput").ap()
    io["w2T"] = nc.dram_tensor("w2T", [2 * D, D], FP, kind="ExternalInput").ap()
    io["bq"] = nc.dram_tensor("bq", [128, 2], FP, kind="ExternalInput").ap()
    io["bk"] = nc.dram_tensor("bk", [128, 2], FP, kind="ExternalInput").ap()
    io["bv"] = nc.dram_tensor("bv", [1, D], FP, kind="ExternalInput").ap()
    io["bm"] = nc.dram_tensor("bm", [128, 2], FP, kind="ExternalInput").ap()
    io["b1"] = nc.dram_tensor("b1", [128, 4], FP, kind="ExternalInput").ap()
    io["b2"] = nc.dram_tensor("b2", [128, 2], FP, kind="ExternalInput").ap()
    io["out"] = nc.dram_tensor("out", [D, NS], FP, kind="ExternalOutput").ap()

    import contextlib
    with tile.TileContext(nc) as tc:
        with contextlib.ExitStack() as es:
            _emit(nc, tc, io, es)
    nc.compile()
    _BUILT["nc"] = nc
    return nc


def _prep_inputs(x, source, Wq, bq, Wk, bk, Wv, bv, Wm, bm, W1, b1, W2, b2):
    perm = np.array([4 * d + h for h in range(H) for d in range(DH)])
    f32 = lambda a: np.ascontiguousarray(a, dtype=np.float32)

    shared = {
        "wqT": f32(Wq[perm, :].T),
        "wkT": f32(Wk[perm, :].T),
        "wvT": f32(Wv[perm, :].T),
        "wmT": f32(Wm[:, perm].T),
        "w1xT": f32(W1.T[0:D, :]),
        "w1mT": f32(W1.T[D:2 * D, :]),
        "w2T": f32(W2.T),
        "bq": f32(bq[perm].reshape(2, 128).T),
        "bk": f32(bk[perm].reshape(2, 128).T),
        "bv": f32(bv[perm].reshape(1, D)),
        "bm": f32(bm.reshape(2, 128).T),
        "b1": f32(b1.reshape(4, 128).T),
        "b2": f32(b2.reshape(2, 128).T),
    }
    in_maps = []
    for core in range(NCORES):
        b, s = core // 4, core % 4
        m = dict(shared)
        m["xs"] = f32(x[b][:, s * NS:(s + 1) * NS])
        m["src"] = f32(source[b])
        in_maps.append(m)
    return in_maps


def run(inputs, **spmd_kwargs):
    """Build (cached), run on cores 0-7, return (full_output, BassKernelResults)."""
    nc = _build()
    in_maps = _prep_inputs(**inputs)
    res = bass_utils.run_bass_kernel_spmd(
        nc, in_maps, core_ids=list(range(NCORES)), **spmd_kwargs)
    full = np.empty((B, D, N), dtype=np.float32)
    for core in range(NCORES):
        b, s = core // 4, core % 4
        full[b][:, s * NS:(s + 1) * NS] = res.results[core]["out"]
    return full, res


def kernel(**inputs):
    full, _ = run(inputs)
    return full
